# revision 15
# baseline (speedup 1.0000x reference)
"""Trainium2 Bass kernel for a dense recurrent scan (nn_CXBPU_55611236549128).

Math (per timestep t, K=4 microsteps):
    inj  = x_t @ W_in.T + b_in                  scattered into sensory_indices
    h    = relu(h @ W_rec.T + scatter(inj))     microstep 0
    h    = relu(h @ W_rec.T)                    microsteps 1..K-1
    out_t = h[:, output_indices] @ W_out.T + b_out

Sharding: data-parallel over batch, 8 rows per core, W_rec replicated.

Per-core design ("scheme K": transpose-free PE stream + DVE block transpose):
  - h kept feature-major: 16 chunk slabs [128 feats (partitions), 8 batch],
    stored in four sub-piece tiles (chunks 0-4 / 5-9 / 10-12 / 13-15).
  - Main matmuls: 4 PE column groups (tile_position=(0,32u)); group u OWNS
    output-feature slice u (512 features) and accumulates ALL 16 k-chunks
    in place into psum partitions [32u:32u+8].  No cross-partition sum and
    no PE transpose-matmuls needed - the PE does nothing but the 8192
    essential moving rows per microstep, in 128x32 col-tiled mode always.
  - psum split in two pieces: A = chunks 0..9 (psum cols 0..320), B =
    chunks 10..15.  Piece A closes at ~2/3 of the microstep so its
    evacuation overlaps piece B's matmuls; piece B's tail overlaps the
    early rounds of the next microstep: no PE bubble.
  - Evacuation: ACT relu+cast (fp32 psum -> fp16), then DVE 32x32 blockwise
    STREAM_TRANSPOSE turns [batch-part, feat] into feature-major h chunks.
    Each piece is evacuated in 2 sub-pieces (separate ev/h tiles) so the
    ACT->DVE chain latency per sub-piece stays well under the PE slack.
    A host-side feature permutation (gmap) makes the block-diagonal
    transpose land exactly in chunk layout:
        feature at (chunk c, partition 32u+p) = u*512 + 32c + p
  - Junk psum partitions (32u+8..32u+32) flow only into junk h columns
    (cols 8..32 of each 32-block) which are never read - no zeroing needed.
  - Injection enters as ONE EXTRA MATMUL round at s==0: stationary e8
    (identity [128,8]) x moving injP (inj in [batch-row, feature] layout,
    rows 8..128 zeroed once) accumulates the scatter into psum directly,
    so every evac is a plain relu and the DVE never touches injections.
  - Readout: 16 tiny matmuls vs scatter-expanded W_out (wsel) spread over
    the 4 column groups (pr psum strips [32i:32i+2]), DVE-combined.
"""

from contextlib import ExitStack

import numpy as np

N = 2048
B = 64
T = 128
NCORES = 8
BPC = B // NCORES  # 8 batch rows per core
NCHUNK = N // 128  # 16
# evac sub-pieces: (col offset, ncols, chunk base, nchunks); the first is
# smallest so the next microstep's round 0 unblocks as early as possible
SUBC = [1, 2, 4, 4, 5]
SUBS = []
_off = 0
for _cn in SUBC:
    SUBS.append((_off * 32, _cn * 32, _off, _cn))
    _off += _cn

_CACHE = {}


def _build_nc(n_steps):
    import concourse.mybir as mybir
    import concourse.tile as tile
    from concourse import bacc

    f32 = mybir.dt.float32
    f16 = mybir.dt.float16
    Relu = mybir.ActivationFunctionType.Relu
    nc = bacc.Bacc(trn_type="TRN2")

    wt_d = nc.dram_tensor("wt", [NCHUNK * 128, N], f16, kind="ExternalInput")
    injp_d = nc.dram_tensor("injp", [n_steps, BPC, N], f16, kind="ExternalInput")
    wsel_d = nc.dram_tensor("wsel", [128, 2 * NCHUNK], f16, kind="ExternalInput")
    e8_d = nc.dram_tensor("e8", [128, BPC], f16, kind="ExternalInput")
    out_d = nc.dram_tensor("out", [2, n_steps * BPC], f32, kind="ExternalOutput")

    NSUB = len(SUBS)
    with tile.TileContext(nc) as tc, ExitStack() as ctx:
        const = ctx.enter_context(tc.tile_pool(name="const", bufs=1))
        hpools = [ctx.enter_context(tc.tile_pool(name=f"h{i}", bufs=2))
                  for i in range(NSUB)]
        epools = [ctx.enter_context(tc.tile_pool(name=f"ev{i}", bufs=2))
                  for i in range(NSUB)]
        apool = ctx.enter_context(tc.tile_pool(name="acc", bufs=2))
        ipool = ctx.enter_context(tc.tile_pool(name="injp", bufs=2))
        ppool = ctx.enter_context(tc.tile_pool(name="ps", bufs=2, space="PSUM"))
        rpool = ctx.enter_context(tc.tile_pool(name="pr", bufs=2, space="PSUM"))

        # resident W^T slabs, one tile per contraction chunk so the first
        # microsteps only wait for the slabs they actually consume
        wts = [const.tile([128, N], f16, name=f"wt{c}") for c in range(NCHUNK)]
        wsel = const.tile([128, 2 * NCHUNK], f16)
        nc.sync.dma_start(wsel[:], wsel_d[:])
        e8 = const.tile([128, BPC], f16)
        nc.sync.dma_start(e8[:], e8_d[:])
        outst = const.tile([2, n_steps * BPC], f32)

        # two injection buffers used round-robin; rows 8..128 zeroed once
        # (the per-timestep DMA only writes rows 0..8, zeros persist) so the
        # e8 matmul never multiplies 0 x garbage.
        injps = [ipool.tile([128, N], f16, name=f"injp{i}") for i in range(2)]
        nc.vector.memset(injps[0][:], 0.0)
        nc.vector.memset(injps[1][:], 0.0)

        tc.strict_bb_all_engine_barrier()

        # t=0 injection first: it must not queue behind the weight slabs
        nc.sync.dma_start(injps[0][0:BPC, :], injp_d[0])

        # 8 MB weight load issued after the barrier so it overlaps the first
        # timestep; slabs arrive roughly in chunk order, matching the round
        # order of the first real matmuls.
        for c in range(NCHUNK):
            eng = (nc.sync, nc.scalar, nc.gpsimd)[c % 3]
            eng.dma_start(wts[c][:], wt_d[c * 128 : (c + 1) * 128, :])

        # chunk c -> (sub-piece index, col offset within it)
        _cmap = {}
        for _i, (_o, _n, _cb, _cn2) in enumerate(SUBS):
            for _c in range(_cb, _cb + _cn2):
                _cmap[_c] = (_i, 32 * (_c - _cb))

        def hslice(h, c):
            i, o = _cmap[c]
            return h[i][:, o : o + BPC]

        def emit_round(h, ps, r, injp, skip_h, first_r):
            # inj round (s==0): psum[32u+b, x] += injp[b, u*512+x]
            if r < 0:
                lh = e8[:, 0:BPC]
            else:
                lh = hslice(h, r)
            last = (r == NCHUNK - 1) if r >= 0 else skip_h
            for u in range(4):
                nc.tensor.matmul(
                    ps[32 * u : 32 * u + BPC, :],
                    lhsT=lh,
                    rhs=(injp[:, u * 512 : (u + 1) * 512] if r < 0
                         else wts[r][:, u * 512 : (u + 1) * 512]),
                    start=first_r,
                    stop=last,
                    tile_position=(0, 32 * u),
                )

        def alloc_evac():
            # allocate all ev/h tiles BEFORE any op of the microstep is
            # emitted: the pool-rotation WAR threshold is conservative
            # (emission-order based), so late allocation would gate ACT's
            # evacs on unrelated just-emitted DVE ops
            evs = [epools[i].tile([128, SUBS[i][1]], f16, name=f"ev{i}")
                   for i in range(NSUB)]
            nhs = [hpools[i].tile([128, SUBS[i][1]], f16, name=f"h{i}")
                   for i in range(NSUB)]
            return evs, nhs

        def emit_evacs(ps, evs, nhs):
            # Relu+cast evacs then DVE blockwise transposes.  Sub-piece 0
            # does both on DVE (no cross-engine handoff) so the next
            # microstep's round 0 unblocks earliest; the rest evac on ACT in
            # parallel with DVE's transposes.  ACT ops are emitted BEFORE any
            # DVE op: the framework's cross-engine WAR thresholds are
            # emission-position conservative (an op waits for every DVE op
            # emitted before it), so ACT-first keeps the evacs gated only on
            # the psum close, not on DVE progress.
            for i in range(1, NSUB):
                off, ncols = SUBS[i][0], SUBS[i][1]
                nc.scalar.activation(evs[i][:], ps[:, off : off + ncols], Relu)
            nc.vector.tensor_relu(evs[0][:], ps[:, 0 : SUBS[0][1]])
            for i in range(NSUB):
                nc.vector.transpose(nhs[i][:], evs[i][:])
            return tuple(nhs)

        def make_readout(h, t):
            def emit():
                pr = rpool.tile([128, BPC], f32, name="pr")
                for i in range(4):
                    for cc in range(4):
                        c = 4 * i + cc
                        nc.tensor.matmul(
                            pr[32 * i : 32 * i + 2, :],
                            lhsT=wsel[:, 2 * c : 2 * c + 2],
                            rhs=hslice(h, c),
                            start=(cc == 0),
                            stop=(cc == 3),
                            tile_position=(0, 32 * i),
                        )
                acc = apool.tile([32, BPC], f32, name="acc")
                nc.vector.tensor_copy(acc[:], pr[0:32, :])
                nc.vector.tensor_add(acc[:], acc[:], pr[32:64, :])
                nc.vector.tensor_add(acc[:], acc[:], pr[64:96, :])
                nc.vector.tensor_add(acc[:], acc[:], pr[96:128, :])
                nc.vector.tensor_copy(outst[:, t * BPC : (t + 1) * BPC], acc[0:2, :])

            return emit

        pend_ro = []
        h = None
        for t in range(n_steps):
            injp = injps[t % 2]
            if t + 1 < n_steps:
                nc.sync.dma_start(injps[(t + 1) % 2][0:BPC, :], injp_d[t + 1])
            for s in range(4):
                first = t == 0 and s == 0
                ip = injp if s == 0 else None
                ps = ppool.tile([128, 512], f32, name="ps")
                evs, nhs = alloc_evac()

                first_r = True
                if ip is not None:
                    emit_round(h, ps, -1, ip, first, True)
                    first_r = False
                if not first:
                    for r in range(NCHUNK):
                        emit_round(h, ps, r, None, False, first_r and r == 0)
                        if r == 4:
                            # previous timestep's readout: its h tiles are
                            # ready and the PE stream has plenty of issue
                            # slack mid-microstep
                            for fn in pend_ro:
                                fn()
                            pend_ro = []
                for fn in pend_ro:
                    fn()
                pend_ro = []
                h = emit_evacs(ps, evs, nhs)

            pend_ro = [make_readout(h, t)]

        for fn in pend_ro:
            fn()
        nc.sync.dma_start(out_d[:], outst[:])
    nc.compile()
    return nc


def _prep_inputs(inputs, W_rec, W_in, b_in, W_out, sensory_indices, output_indices,
                 n_steps):
    inputs = np.asarray(inputs, np.float32)
    W_rec = np.asarray(W_rec, np.float32)
    W_in = np.asarray(W_in, np.float32)
    b_in = np.asarray(b_in, np.float32)
    W_out = np.asarray(W_out, np.float32)
    sens = np.asarray(sensory_indices).astype(np.int64)
    oidx = np.asarray(output_indices).astype(np.int64)

    # feature at (chunk c, partition 32u+p) = u*512 + 32c + p
    part = np.arange(128)
    gmap = ((part[None, :] // 32) * 512 + 32 * np.arange(NCHUNK)[:, None]
            + (part[None, :] % 32))  # [16, 128]

    wtf = np.ascontiguousarray(W_rec.T)  # [f_in, f_out]
    wt = np.ascontiguousarray(wtf[gmap.reshape(-1), :].astype(np.float16))

    wsel_full = np.zeros((2, N), np.float32)
    np.add.at(wsel_full, (slice(None), oidx), W_out)
    wsel = np.ascontiguousarray(
        wsel_full[:, gmap].transpose(2, 1, 0).reshape(128, 2 * NCHUNK)
        .astype(np.float16))

    e8 = np.zeros((128, BPC), np.float16)
    e8[np.arange(BPC), np.arange(BPC)] = 1.0

    # injection in [batch-row, feature] (psum) layout, per core
    inj_all = inputs[:, :n_steps, :] @ W_in.T + b_in  # [B, T, 256]
    inj_dense = np.zeros((B, n_steps, N), np.float32)
    np.add.at(inj_dense, (slice(None), slice(None), sens), inj_all)
    injp_cores = []
    for g in range(NCORES):
        a = inj_dense[g * BPC : (g + 1) * BPC]  # [8, T, 2048]
        injp_cores.append(np.ascontiguousarray(
            a.transpose(1, 0, 2).astype(np.float16)))  # [T, 8, 2048]

    return wt, injp_cores, wsel, e8


def _run(inputs, W_rec, W_in, b_in, W_out, b_out, sensory_indices, output_indices,
         K, n_steps=T, trace=False):
    from concourse.bass_utils import run_bass_kernel_spmd

    assert int(K) == 4
    wt, injp_cores, wsel, e8 = _prep_inputs(
        inputs, W_rec, W_in, b_in, W_out, sensory_indices, output_indices, n_steps)

    if n_steps not in _CACHE:
        _CACHE[n_steps] = _build_nc(n_steps)
    nc = _CACHE[n_steps]

    in_maps = [
        {"wt": wt, "injp": injp_cores[g], "wsel": wsel, "e8": e8}
        for g in range(NCORES)
    ]
    res = run_bass_kernel_spmd(nc, in_maps, list(range(NCORES)), trace=trace)

    b_out = np.asarray(b_out, np.float32)
    outs = []
    for g in range(NCORES):
        r = np.asarray(res.results[g]["out"])  # [2, T*8]
        outs.append(r.reshape(2, n_steps, BPC).transpose(2, 1, 0))  # [8, T, 2]
    full = np.concatenate(outs, axis=0) + b_out  # [B, T, 2]
    return np.ascontiguousarray(full.astype(np.float32)), res


def kernel(**inputs):
    out, _ = _run(
        inputs["inputs"], inputs["W_rec"], inputs["W_in"], inputs["b_in"],
        inputs["W_out"], inputs["b_out"], inputs["sensory_indices"],
        inputs["output_indices"], inputs["K"],
    )
    return out
